# revision 1
# baseline (speedup 1.0000x reference)
"""3-layer GCN encoder (CGCNN-style) on 8 Trainium2 NeuronCores.

Sharding: nodes (and their incident in-edges, plus self-loops) are
partitioned across 8 cores; the 128x128 weights are replicated; the
transformed features are AllGathered each layer to serve as the gather
table; BatchNorm statistics are AllReduced; per-graph pooled partial
sums are computed on-device per core and summed on the host at unshard.

Self-contained: only numpy + the concourse (bass) toolchain.
"""
import numpy as np
import ml_dtypes

import concourse.bass as bass
import concourse.bacc as bacc
import concourse.mybir as mybir
import concourse.tile as tile
from concourse import library_config
from concourse.bass_utils import run_bass_kernel_spmd

FP = mybir.dt.float32
BF = mybir.dt.bfloat16
I16 = mybir.dt.int16
BF_NP = np.dtype(ml_dtypes.bfloat16)
EPS = 1e-5


class Cfg:
    def __init__(self, N=50000, E=800000, G=64, n_cores=8, half=32768,
                 batch_chunks=64):
        self.N, self.E, self.G, self.n_cores = N, E, G, n_cores
        self.HALF = half
        self.SL = N // n_cores          # nodes per core
        assert self.SL * n_cores == N
        self.NT = (self.SL + 127) // 128  # dst tiles per core
        self.SLP = self.NT * 128          # padded slice length
        self.BATCH_CH = batch_chunks      # gather batch size in 128-chunks


DEFAULT_CFG = Cfg()


# --------------------------------------------------------------------------
# Host-side preprocessing: graph partitioning and operand packing
# --------------------------------------------------------------------------

def host_prep(cfg, x, edge_index, batch, W1, W2, W3, g1, be1, g2, be2, g3, be3):
    N, G, SL, NT = cfg.N, cfg.G, cfg.SL, cfg.NT
    src = np.asarray(edge_index[0], dtype=np.int64)
    dst = np.asarray(edge_index[1], dtype=np.int64)
    deg = np.bincount(dst, minlength=N).astype(np.float64) + 1.0
    dinv = 1.0 / np.sqrt(deg)
    a_src = np.concatenate([src, np.arange(N)])
    a_dst = np.concatenate([dst, np.arange(N)])
    a_nrm = np.concatenate([dinv[src] * dinv[dst], dinv * dinv]).astype(np.float32)

    percore = []
    counts = np.zeros((cfg.n_cores, NT, 2), np.int64)
    for c in range(cfg.n_cores):
        sel = (a_dst // SL) == c
        s_ = a_src[sel]
        d_ = a_dst[sel] - SL * c
        n_ = a_nrm[sel]
        tile_id = d_ // 128
        half = (s_ >= cfg.HALF).astype(np.int64)
        np.add.at(counts[c], (tile_id, half), 1)
        percore.append((s_, d_, n_, tile_id, half))

    # shared chunk structure: chunks per (tile, half) = max over cores
    nch = np.maximum(np.ceil(counts.max(axis=0) / 128).astype(np.int64), 1)
    K = int(nch.sum()) * 128            # shared padded edge-stream length

    # chunk order: half-major, then tile
    # meta per half: list of (tile, n_chunks); plus batch splits
    meta = {"nch": nch, "K": K, "halves": []}
    gchunk = 0
    for h in (0, 1):
        tiles = [(t, int(nch[t, h])) for t in range(NT)]
        total_ch = sum(n for _, n in tiles)
        # flat chunk -> (tile, first, last)
        flat = []
        for t, n in tiles:
            for j in range(n):
                flat.append((t, j == 0, j == n - 1))
        # batches of <= BATCH_CH chunks
        batches = []
        pos = 0
        while pos < total_ch:
            n = min(cfg.BATCH_CH, total_ch - pos)
            batches.append((gchunk + pos, pos, n))  # (global chunk idx, local, n)
            pos += n
        meta["halves"].append({
            "tiles": tiles, "flat": flat, "batches": batches,
            "chunk0": gchunk, "n_chunks": total_ch,
        })
        gchunk += total_ch
    meta["n_chunks_total"] = gchunk

    # per-core packed arrays
    in_maps = []
    Wcat = np.concatenate([np.asarray(W1), np.asarray(W2), np.asarray(W3)],
                          axis=1).astype(BF_NP)          # [128, 384]
    bncat = np.stack([np.asarray(v, np.float32) for v in
                      (g1, be1, g2, be2, g3, be3)], axis=1)  # [128, 6]
    ident = np.eye(128, dtype=BF_NP)
    xT = np.ascontiguousarray(np.asarray(x, np.float32).T)   # [128, N]
    batch_np = np.asarray(batch, np.int64)

    for c in range(cfg.n_cores):
        s_, d_, n_, tile_id, half = percore[c]
        idx_stream = np.zeros(K, np.int16)
        nrm_stream = np.zeros(K, np.float32)
        dl_stream = np.zeros(K, np.int64)
        pos = 0
        for h in (0, 1):
            for t in range(NT):
                m = (half == h) & (tile_id == t)
                cnt = int(m.sum())
                room = int(nch[t, h]) * 128
                assert cnt <= room
                idx_stream[pos:pos + cnt] = (s_[m] - cfg.HALF * h).astype(np.int16)
                nrm_stream[pos:pos + cnt] = n_[m]
                dl_stream[pos:pos + cnt] = d_[m] % 128
                pos += room
        assert pos == K
        # S: [K,128] -> [128, K/128, 128]
        S = np.zeros((K, 128), np.float32)
        S[np.arange(K), dl_stream] = nrm_stream
        S = np.ascontiguousarray(
            S.reshape(K // 128, 128, 128).transpose(1, 0, 2)).astype(BF_NP)
        # idx: [128, K/16] replicated into the 8 gpsimd core groups
        idx_t = np.zeros((128, K // 16), np.int16)
        w = idx_stream.reshape(K // 16, 16).T
        for k in range(8):
            idx_t[16 * k:16 * (k + 1), :] = w
        # pooling one-hot P: [128, NT*G]
        P = np.zeros((128, NT * G), np.float32)
        for t in range(NT):
            base = SL * c + t * 128
            nvalid = min(128, SL - t * 128)
            gids = batch_np[base:base + nvalid]
            P[np.arange(nvalid), t * G + gids] = 1.0
        # xT slice for this core, padded to SLP cols
        xs = np.zeros((128, cfg.SLP), BF_NP)
        xs[:, :SL] = xT[:, SL * c:SL * (c + 1)].astype(BF_NP)
        in_maps.append({
            "xTs": xs, "idx": idx_t, "S": S, "P": P.astype(BF_NP),
            "Wc": Wcat, "bn": bncat, "ident": ident,
        })
    return meta, in_maps


# --------------------------------------------------------------------------
# Kernel builder (one SPMD program; per-core differences live in the data)
# --------------------------------------------------------------------------

def build_gcn(cfg, meta, reps=1, no_coll=False, no_gather=False):
    N, G, SL, NT, SLP = cfg.N, cfg.G, cfg.SL, cfg.NT, cfg.SLP
    K = meta["K"]
    nc = bacc.Bacc("TRN2", target_bir_lowering=False, debug=False,
                   num_devices=cfg.n_cores)
    xTs_d = nc.dram_tensor("xTs", [128, SLP], BF, kind="ExternalInput")
    idx_d = nc.dram_tensor("idx", [128, K // 16], I16, kind="ExternalInput")
    S_d = nc.dram_tensor("S", [128, K // 128, 128], BF, kind="ExternalInput")
    P_d = nc.dram_tensor("P", [128, NT * G], BF, kind="ExternalInput")
    W_d = nc.dram_tensor("Wc", [128, 384], BF, kind="ExternalInput")
    bn_d = nc.dram_tensor("bn", [128, 6], FP, kind="ExternalInput")
    id_d = nc.dram_tensor("ident", [128, 128], BF, kind="ExternalInput")
    out_d = nc.dram_tensor("out", [G, 128], FP, kind="ExternalOutput")

    groups = [list(range(cfg.n_cores))]
    shared = "Shared" if cfg.n_cores > 4 else "Local"

    with tile.TileContext(nc) as tc:
        with (
            tc.tile_pool(name="const", bufs=1) as cp,
            tc.tile_pool(name="hbuf", bufs=1) as hp,
            tc.tile_pool(name="acc", bufs=1) as accp,
            tc.tile_pool(name="gbuf", bufs=2) as gp,
            tc.tile_pool(name="sbufS", bufs=2) as sp,
            tc.tile_pool(name="msg", bufs=6) as msp,
            tc.tile_pool(name="obuf", bufs=3) as obp,
            tc.tile_pool(name="small", bufs=8) as smp,
            tc.tile_pool(name="psum", bufs=1, space="PSUM") as psp,
            tc.tile_pool(name="dram", bufs=1, space="DRAM") as dram,
        ):
            idx_t = cp.tile([128, K // 16], I16)
            nc.sync.dma_start(idx_t[:], idx_d[:])
            W_t = cp.tile([128, 384], BF)
            nc.sync.dma_start(W_t[:], W_d[:])
            bn_t = cp.tile([128, 6], FP)
            nc.sync.dma_start(bn_t[:], bn_d[:])
            id_t = cp.tile([128, 128], BF)
            nc.sync.dma_start(id_t[:], id_d[:])
            P_t = cp.tile([128, NT * G], BF)
            nc.sync.dma_start(P_t[:], P_d[:])
            x_t = hp.tile([128, SLP], BF)
            nc.sync.dma_start(x_t[:], xTs_d[:])

            eps_t = cp.tile([128, 1], FP)
            nc.gpsimd.memset(eps_t[:], EPS)
            accum = accp.tile([128, SLP], FP)
            sqdump = accp.tile([128, SLP], FP)

            for rep in range(reps):
                h_cur = x_t
                for l in range(3):
                    # ---- hW slice: [node, feat] blocks -> AG input ----
                    agin = dram.tile([SL, 128], BF, tag="agin", bufs=2)
                    nblk = (NT + 3) // 4
                    for blk in range(nblk):
                        c0 = blk * 4
                        nch_blk = min(4, NT - c0)
                        ps = psp.tile([128, 512], FP, tag="pshw", bufs=2,
                                      name=f"pshw{l}_{blk}_{rep}")
                        for j in range(nch_blk):
                            t_ = c0 + j
                            nc.tensor.matmul(
                                ps[:, j * 128:(j + 1) * 128],
                                h_cur[:, t_ * 128:(t_ + 1) * 128],
                                W_t[:, l * 128:(l + 1) * 128],
                                start=True, stop=True,
                            )
                        ob = obp.tile([128, 512], BF, tag="ob")
                        nc.scalar.copy(ob[:, :nch_blk * 128],
                                       ps[:, :nch_blk * 128])
                        r0 = c0 * 128
                        r1 = min(r0 + nch_blk * 128, SL)
                        nfull = (r1 - r0) // 128
                        if nfull > 0:
                            nc.sync.dma_start(
                                agin[r0:r0 + nfull * 128, :]
                                .rearrange("(j p) f -> p j f", p=128),
                                ob[:, :nfull * 128]
                                .rearrange("p (j f) -> p j f", f=128))
                        rem = (r1 - r0) - nfull * 128
                        if rem > 0:
                            nc.sync.dma_start(
                                agin[r0 + nfull * 128:r1, :],
                                ob[:rem, nfull * 128:(nfull + 1) * 128])
                    # ---- AllGather -> full table ----
                    T = dram.tile([N, 128], BF, addr_space=shared,
                                  tag="T", bufs=2)
                    if no_coll:
                        nc.sync.dma_start(T[0:SL, :], agin[:])
                    else:
                        nc.gpsimd.collective_compute(
                            "AllGather", mybir.AluOpType.bypass,
                            replica_groups=groups,
                            ins=[agin.opt()], outs=[T.opt()],
                        )
                    # ---- gather + scatter-matmul ----
                    evac_done = [False] * NT
                    psc_live = {}
                    if no_gather:
                        nc.gpsimd.memset(accum[:], 0.0)
                    for h in ((), (0, 1))[0 if no_gather else 1]:
                        hm = meta["halves"][h]
                        base = T[0:cfg.HALF, :] if h == 0 else T[cfg.HALF:N, :]
                        for (gc0, lc0, nchb) in hm["batches"]:
                            nidx = nchb * 128
                            g = gp.tile([128, 1, cfg.BATCH_CH * 128], BF,
                                        tag="g")
                            nc.gpsimd.dma_gather(
                                g[:, :, :nidx], base,
                                idx_t[:, gc0 * 8:(gc0 + nchb) * 8],
                                nidx, nidx, 128,
                                transpose=True, single_packet=False,
                            )
                            st = sp.tile([128, cfg.BATCH_CH, 128], BF,
                                         tag="st")
                            nc.sync.dma_start(
                                st[:, :nchb, :], S_d[:, gc0:gc0 + nchb, :])
                            for cc in range(nchb):
                                t_, first, last = hm["flat"][lc0 + cc]
                                tp = psp.tile([128, 128], BF, tag="tp", bufs=2)
                                nc.tensor.transpose(
                                    tp[:], g[:, 0, cc * 128:(cc + 1) * 128],
                                    id_t[:])
                                ms = msp.tile([128, 128], BF, tag="ms")
                                if cc % 2 == 0:
                                    nc.scalar.copy(ms[:], tp[:])
                                else:
                                    nc.vector.tensor_copy(ms[:], tp[:])
                                if first:
                                    psc_live[t_] = psp.tile(
                                        [128, 128], FP, tag="sc", bufs=3,
                                        name=f"sc{l}_{h}_{t_}_{rep}")
                                psc = psc_live[t_]
                                nc.tensor.matmul(
                                    psc[:], ms[:], st[:, cc, :],
                                    start=first, stop=last,
                                )
                                if last:
                                    dsl = accum[:, t_ * 128:(t_ + 1) * 128]
                                    if not evac_done[t_]:
                                        nc.scalar.copy(dsl, psc[:])
                                        evac_done[t_] = True
                                    else:
                                        nc.vector.tensor_add(dsl, dsl, psc[:])
                    # ---- BN stats + AllReduce ----
                    sums = smp.tile([128, 2], FP, tag="sums")
                    nc.vector.tensor_reduce(
                        sums[:, 0:1], accum[:, :SL],
                        axis=mybir.AxisListType.X, op=mybir.AluOpType.add)
                    nc.vector.tensor_mul(sqdump[:, :SL], accum[:, :SL],
                                         accum[:, :SL])
                    nc.vector.tensor_reduce(sums[:, 1:2], sqdump[:, :SL],
                                            axis=mybir.AxisListType.X,
                                            op=mybir.AluOpType.add)
                    arin = dram.tile([128, 2], FP, tag="arin", bufs=2)
                    arout = dram.tile([128, 2], FP, addr_space=shared,
                                      tag="arout", bufs=2)
                    nc.sync.dma_start(arin[:], sums[:])
                    if no_coll:
                        nc.sync.dma_start(arout[:], arin[:])
                    else:
                        nc.gpsimd.collective_compute(
                            "AllReduce", mybir.AluOpType.add,
                            replica_groups=groups,
                            ins=[arin.opt()], outs=[arout.opt()],
                        )
                    gsums = smp.tile([128, 2], FP, tag="gsums")
                    nc.sync.dma_start(gsums[:], arout[:])
                    # ---- BN affine params ----
                    m = smp.tile([128, 1], FP, tag="m")
                    nc.scalar.mul(m[:], gsums[:, 0:1], 1.0 / N)
                    ex2 = smp.tile([128, 1], FP, tag="ex2")
                    nc.scalar.mul(ex2[:], gsums[:, 1:2], 1.0 / N)
                    var = smp.tile([128, 1], FP, tag="var")
                    nc.vector.tensor_mul(var[:], m[:], m[:])
                    nc.vector.tensor_sub(var[:], ex2[:], var[:])
                    sd = smp.tile([128, 1], FP, tag="sd")
                    nc.scalar.activation(sd[:], var[:],
                                         mybir.ActivationFunctionType.Sqrt,
                                         bias=eps_t[:])
                    inv = smp.tile([128, 1], FP, tag="inv")
                    nc.vector.reciprocal(inv[:], sd[:])
                    sc_l = smp.tile([128, 1], FP, tag="scl")
                    nc.vector.tensor_mul(sc_l[:], inv[:],
                                         bn_t[:, 2 * l:2 * l + 1])
                    bi_l = smp.tile([128, 1], FP, tag="bil")
                    nc.vector.tensor_mul(bi_l[:], m[:], sc_l[:])
                    nc.vector.tensor_sub(bi_l[:], bn_t[:, 2 * l + 1:2 * l + 2],
                                         bi_l[:])
                    # ---- sigmoid -> next h ----
                    h_next = hp.tile([128, SLP], BF, tag="h", bufs=2)
                    nc.scalar.activation(h_next[:], accum[:],
                                         mybir.ActivationFunctionType.Sigmoid,
                                         bias=bi_l[:], scale=sc_l[:])
                    h_cur = h_next
                # ---- pooling ----
                pps = psp.tile([G, 128], FP, tag="pool", bufs=1)
                for t in range(NT):
                    tp = psp.tile([128, 128], BF, tag="tp", bufs=2)
                    nc.tensor.transpose(
                        tp[:], h_cur[:, t * 128:(t + 1) * 128], id_t[:])
                    hn = msp.tile([128, 128], BF, tag="ms")
                    if t % 2 == 0:
                        nc.scalar.copy(hn[:], tp[:])
                    else:
                        nc.vector.tensor_copy(hn[:], tp[:])
                    nc.tensor.matmul(
                        pps[:], P_t[:, t * G:(t + 1) * G], hn[:],
                        start=(t == 0), stop=(t == NT - 1),
                    )
                po = smp.tile([G, 128], FP, tag="po")
                nc.vector.tensor_copy(po[:], pps[:])
                nc.sync.dma_start(out_d[:], po[:])
    nc.compile()
    return nc


# --------------------------------------------------------------------------
# Entry point
# --------------------------------------------------------------------------

def kernel(**inputs):
    cfg = DEFAULT_CFG
    x = np.asarray(inputs["x"], np.float32)
    edge_index = np.asarray(inputs["edge_index"])
    batch = np.asarray(inputs["batch"])
    args = [x, edge_index, batch] + [
        np.asarray(inputs[k], np.float32) for k in
        ("W1", "W2", "W3", "g1", "be1", "g2", "be2", "g3", "be3")]
    meta, in_maps = host_prep(cfg, *args)
    nc = build_gcn(cfg, meta, reps=1)
    res = run_bass_kernel_spmd(nc, in_maps, core_ids=list(range(cfg.n_cores)))
    pooled = np.zeros((cfg.G, 128), np.float64)
    for c in range(cfg.n_cores):
        pooled += res.results[c]["out"].astype(np.float64)
    cnt = np.bincount(np.asarray(batch, np.int64), minlength=cfg.G).astype(np.float64)
    out = pooled / np.maximum(cnt, 1.0)[:, None]
    return out.astype(np.float32)



# revision 18
# speedup vs baseline: 7.5636x; 7.5636x over previous
"""3-layer GCN encoder (CGCNN-style) on 8 Trainium2 NeuronCores.

Sharding: nodes (and their incident in-edges, plus self-loops) are
partitioned across 8 cores; the 128x128 weights are replicated; the
transformed features are AllGathered each layer to serve as the gather
table; BatchNorm statistics are AllReduced; per-graph pooled partial
sums are computed on-device per core and summed on the host at unshard.

Self-contained: only numpy + the concourse (bass) toolchain.
"""
import numpy as np
import ml_dtypes

import concourse.bass as bass
import concourse.bacc as bacc
import concourse.mybir as mybir
import concourse.tile as tile
from concourse import library_config
from concourse.bass_utils import run_bass_kernel_spmd

FP = mybir.dt.float32
BF = mybir.dt.bfloat16
I16 = mybir.dt.int16
BF_NP = np.dtype(ml_dtypes.bfloat16)
EPS = 1e-5


class Cfg:
    def __init__(self, N=50000, E=800000, G=64, n_cores=8, half=32768,
                 batch_chunks=32):
        self.N, self.E, self.G, self.n_cores = N, E, G, n_cores
        self.HALF = half
        self.SL = N // n_cores          # nodes per core
        assert self.SL * n_cores == N
        self.NT = (self.SL + 127) // 128  # dst tiles per core
        self.SLP = self.NT * 128          # padded slice length
        self.BATCH_CH = batch_chunks      # gather batch size in 128-chunks


DEFAULT_CFG = Cfg()


# --------------------------------------------------------------------------
# Host-side preprocessing: graph partitioning and operand packing
# --------------------------------------------------------------------------

def host_prep(cfg, x, edge_index, batch, W1, W2, W3, g1, be1, g2, be2, g3, be3):
    N, G, SL, NT = cfg.N, cfg.G, cfg.SL, cfg.NT
    src = np.asarray(edge_index[0], dtype=np.int64)
    dst = np.asarray(edge_index[1], dtype=np.int64)
    deg = np.bincount(dst, minlength=N).astype(np.float64) + 1.0
    dinv = 1.0 / np.sqrt(deg)
    a_src = np.concatenate([src, np.arange(N)])
    a_dst = np.concatenate([dst, np.arange(N)])
    a_nrm = np.concatenate([dinv[src] * dinv[dst], dinv * dinv]).astype(np.float32)

    percore = []
    counts = np.zeros((cfg.n_cores, NT, 2), np.int64)
    for c in range(cfg.n_cores):
        sel = (a_dst // SL) == c
        s_ = a_src[sel]
        d_ = a_dst[sel] - SL * c
        n_ = a_nrm[sel]
        tile_id = d_ // 128
        half = (s_ >= cfg.HALF).astype(np.int64)
        np.add.at(counts[c], (tile_id, half), 1)
        percore.append((s_, d_, n_, tile_id, half))

    # shared chunk structure: chunks per (tile, half) = max over cores
    nch = np.maximum(np.ceil(counts.max(axis=0) / 128).astype(np.int64), 1)
    K = int(nch.sum()) * 128            # shared padded edge-stream length

    # chunk order: half-major, then tile
    # meta per half: list of (tile, n_chunks); plus batch splits
    meta = {"nch": nch, "K": K, "halves": []}
    gchunk = 0
    for h in (0, 1):
        tiles = [(t, int(nch[t, h])) for t in range(NT)]
        total_ch = sum(n for _, n in tiles)
        # flat chunk -> (tile, first, last)
        flat = []
        for t, n in tiles:
            for j in range(n):
                flat.append((t, j == 0, j == n - 1))
        # batches of <= BATCH_CH chunks
        batches = []
        pos = 0
        while pos < total_ch:
            n = min(cfg.BATCH_CH, total_ch - pos)
            batches.append((gchunk + pos, pos, n))  # (global chunk idx, local, n)
            pos += n
        meta["halves"].append({
            "tiles": tiles, "flat": flat, "batches": batches,
            "chunk0": gchunk, "n_chunks": total_ch,
        })
        gchunk += total_ch
    meta["n_chunks_total"] = gchunk

    # per-core packed arrays
    in_maps = []
    Wcat = np.concatenate([np.asarray(W1), np.asarray(W2), np.asarray(W3)],
                          axis=1).astype(BF_NP)          # [128, 384]
    bncat = np.stack([np.asarray(v, np.float32) for v in
                      (g1, be1, g2, be2, g3, be3)], axis=1)  # [128, 6]
    ident = np.eye(128, dtype=BF_NP)
    xT = np.ascontiguousarray(np.asarray(x, np.float32).T)   # [128, N]
    batch_np = np.asarray(batch, np.int64)

    for c in range(cfg.n_cores):
        s_, d_, n_, tile_id, half = percore[c]
        idx_stream = np.zeros(K, np.int16)
        nrm_stream = np.zeros(K, np.float32)
        dl_stream = np.zeros(K, np.int64)
        pos = 0
        for h in (0, 1):
            for t in range(NT):
                m = (half == h) & (tile_id == t)
                cnt = int(m.sum())
                room = int(nch[t, h]) * 128
                assert cnt <= room
                idx_stream[pos:pos + cnt] = (s_[m] - cfg.HALF * h).astype(np.int16)
                nrm_stream[pos:pos + cnt] = n_[m]
                dl_stream[pos:pos + cnt] = d_[m] % 128
                pos += room
        assert pos == K
        # S: [K,128] -> [128, K/128, 128]
        S = np.zeros((K, 128), np.float32)
        S[np.arange(K), dl_stream] = nrm_stream
        S = np.ascontiguousarray(
            S.reshape(K // 128, 128, 128).transpose(1, 0, 2)).astype(BF_NP)
        # idx: [128, K/16] replicated into the 8 gpsimd core groups
        idx_t = np.zeros((128, K // 16), np.int16)
        w = idx_stream.reshape(K // 16, 16).T
        for k in range(8):
            idx_t[16 * k:16 * (k + 1), :] = w
        # pooling one-hot P: [128, NT*G]
        P = np.zeros((128, NT * G), np.float32)
        for t in range(NT):
            base = SL * c + t * 128
            nvalid = min(128, SL - t * 128)
            gids = batch_np[base:base + nvalid]
            P[np.arange(nvalid), t * G + gids] = 1.0
        # xT slice for this core, padded to SLP cols
        xs = np.zeros((128, cfg.SLP), BF_NP)
        xs[:, :SL] = xT[:, SL * c:SL * (c + 1)].astype(BF_NP)
        in_maps.append({
            "xTs": xs, "idx": idx_t, "S": S, "P": P.astype(BF_NP),
            "Wc": Wcat, "bn": bncat, "ident": ident,
        })
    return meta, in_maps


# --------------------------------------------------------------------------
# Kernel builder (one SPMD program; per-core differences live in the data)
# --------------------------------------------------------------------------

def build_gcn(cfg, meta, reps=1, no_coll=False, no_gather=False):
    N, G, SL, NT, SLP = cfg.N, cfg.G, cfg.SL, cfg.NT, cfg.SLP
    K = meta["K"]
    nc = bacc.Bacc("TRN2", target_bir_lowering=False, debug=False,
                   num_devices=cfg.n_cores, num_swdge_queues=4)
    xTs_d = nc.dram_tensor("xTs", [128, SLP], BF, kind="ExternalInput")
    idx_d = nc.dram_tensor("idx", [128, K // 16], I16, kind="ExternalInput")
    S_d = nc.dram_tensor("S", [128, K // 128, 128], BF, kind="ExternalInput")
    P_d = nc.dram_tensor("P", [128, NT * G], BF, kind="ExternalInput")
    W_d = nc.dram_tensor("Wc", [128, 384], BF, kind="ExternalInput")
    bn_d = nc.dram_tensor("bn", [128, 6], FP, kind="ExternalInput")
    id_d = nc.dram_tensor("ident", [128, 128], BF, kind="ExternalInput")
    out_d = nc.dram_tensor("out", [G, 128], FP, kind="ExternalOutput")

    groups = [list(range(cfg.n_cores))]
    shared = "Shared" if cfg.n_cores > 4 else "Local"

    with tile.TileContext(nc) as tc:
        with (
            tc.tile_pool(name="const", bufs=1) as cp,
            tc.tile_pool(name="hbuf", bufs=1) as hp,
            tc.tile_pool(name="acc", bufs=1) as accp,
            tc.tile_pool(name="gbuf", bufs=5) as gp,
            tc.tile_pool(name="sbufS", bufs=3) as sp,
            tc.tile_pool(name="msg", bufs=6) as msp,
            tc.tile_pool(name="obuf", bufs=3) as obp,
            tc.tile_pool(name="small", bufs=8) as smp,
            tc.tile_pool(name="psum", bufs=1, space="PSUM") as psp,
            tc.tile_pool(name="dram", bufs=1, space="DRAM") as dram,
        ):
            idx_t = cp.tile([128, K // 16], I16)
            nc.sync.dma_start(idx_t[:], idx_d[:])
            W_t = cp.tile([128, 384], BF)
            nc.sync.dma_start(W_t[:], W_d[:])
            bn_t = cp.tile([128, 6], FP)
            nc.sync.dma_start(bn_t[:], bn_d[:])
            id_t = cp.tile([128, 128], BF)
            nc.sync.dma_start(id_t[:], id_d[:])
            P_t = cp.tile([128, NT * G], BF)
            nc.sync.dma_start(P_t[:], P_d[:])
            x_t = hp.tile([128, SLP], BF)
            nc.sync.dma_start(x_t[:], xTs_d[:])

            eps_t = cp.tile([128, 1], FP)
            nc.gpsimd.memset(eps_t[:], EPS)
            accum = accp.tile([128, SLP], FP)
            sqdump = accp.tile([128, SLP], FP)

            for rep in range(reps):
                h_cur = x_t
                for l in range(3):
                    # ---- hW slice: [node, feat] blocks -> AG input ----
                    agin = dram.tile([SL, 128], BF, tag="agin", bufs=2)
                    nblk = (NT + 3) // 4
                    for blk in range(nblk):
                        c0 = blk * 4
                        nch_blk = min(4, NT - c0)
                        ps = psp.tile([128, 512], FP, tag="pshw", bufs=2,
                                      name=f"pshw{l}_{blk}_{rep}")
                        for j in range(nch_blk):
                            t_ = c0 + j
                            nc.tensor.matmul(
                                ps[:, j * 128:(j + 1) * 128],
                                h_cur[:, t_ * 128:(t_ + 1) * 128],
                                W_t[:, l * 128:(l + 1) * 128],
                                start=True, stop=True,
                            )
                        ob = obp.tile([128, 512], BF, tag="ob")
                        nc.scalar.copy(ob[:, :nch_blk * 128],
                                       ps[:, :nch_blk * 128])
                        r0 = c0 * 128
                        r1 = min(r0 + nch_blk * 128, SL)
                        nfull = (r1 - r0) // 128
                        if nfull > 0:
                            nc.sync.dma_start(
                                agin[r0:r0 + nfull * 128, :]
                                .rearrange("(j p) f -> p j f", p=128),
                                ob[:, :nfull * 128]
                                .rearrange("p (j f) -> p j f", f=128))
                        rem = (r1 - r0) - nfull * 128
                        if rem > 0:
                            nc.sync.dma_start(
                                agin[r0 + nfull * 128:r1, :],
                                ob[:rem, nfull * 128:(nfull + 1) * 128])
                    # ---- AllGather -> full table ----
                    T = dram.tile([N, 128], BF, addr_space=shared,
                                  tag="T", bufs=2)
                    if no_coll:
                        nc.sync.dma_start(T[0:SL, :], agin[:])
                    else:
                        nc.gpsimd.collective_compute(
                            "AllGather", mybir.AluOpType.bypass,
                            replica_groups=groups,
                            ins=[agin.opt()], outs=[T.opt()],
                        )
                    # ---- gather + scatter-matmul ----
                    # transpose=False gather lands rows as [edge(part),
                    # chunk, feat] — directly the stationary operand of the
                    # scatter matmul psc[feat,dst] += g_c.T @ S_c.
                    evac_done = [False] * NT
                    psc_live = {}
                    gq = 0
                    if no_gather:
                        nc.gpsimd.memset(accum[:], 0.0)
                    for h in ((), (0, 1))[0 if no_gather else 1]:
                        hm = meta["halves"][h]
                        base = T[0:cfg.HALF, :] if h == 0 else T[cfg.HALF:N, :]
                        for (gc0, lc0, nchb) in hm["batches"]:
                            nidx = nchb * 128
                            g = gp.tile([128, cfg.BATCH_CH, 128], BF,
                                        tag="g")
                            nc.gpsimd.dma_gather(
                                g[:, :nchb, :], base,
                                idx_t[:, gc0 * 8:(gc0 + nchb) * 8],
                                nidx, nidx, 128,
                                transpose=False, single_packet=False,
                                queue_num=gq % 4,
                            )
                            gq += 1
                            st = sp.tile([128, cfg.BATCH_CH, 128], BF,
                                         tag="st")
                            nc.sync.dma_start(
                                st[:, :nchb, :], S_d[:, gc0:gc0 + nchb, :])
                            for cc in range(nchb):
                                t_, first, last = hm["flat"][lc0 + cc]
                                if first:
                                    psc_live[t_] = psp.tile(
                                        [128, 128], FP, tag="sc", bufs=3,
                                        name=f"sc{l}_{h}_{t_}_{rep}")
                                psc = psc_live[t_]
                                nc.tensor.matmul(
                                    psc[:], g[:, cc, :], st[:, cc, :],
                                    start=first, stop=last,
                                )
                                if last:
                                    dsl = accum[:, t_ * 128:(t_ + 1) * 128]
                                    if not evac_done[t_]:
                                        nc.scalar.copy(dsl, psc[:])
                                        evac_done[t_] = True
                                    else:
                                        nc.vector.tensor_add(dsl, dsl, psc[:])
                    # ---- BN stats + AllReduce ----
                    sums = smp.tile([128, 2], FP, tag="sums")
                    nc.vector.tensor_reduce(
                        sums[:, 0:1], accum[:, :SL],
                        axis=mybir.AxisListType.X, op=mybir.AluOpType.add)
                    nc.vector.tensor_mul(sqdump[:, :SL], accum[:, :SL],
                                         accum[:, :SL])
                    nc.vector.tensor_reduce(sums[:, 1:2], sqdump[:, :SL],
                                            axis=mybir.AxisListType.X,
                                            op=mybir.AluOpType.add)
                    arin = dram.tile([128, 2], FP, tag="arin", bufs=2)
                    arout = dram.tile([128, 2], FP, addr_space=shared,
                                      tag="arout", bufs=2)
                    nc.sync.dma_start(arin[:], sums[:])
                    if no_coll:
                        nc.sync.dma_start(arout[:], arin[:])
                    else:
                        nc.gpsimd.collective_compute(
                            "AllReduce", mybir.AluOpType.add,
                            replica_groups=groups,
                            ins=[arin.opt()], outs=[arout.opt()],
                        )
                    gsums = smp.tile([128, 2], FP, tag="gsums")
                    nc.sync.dma_start(gsums[:], arout[:])
                    # ---- BN affine params ----
                    m = smp.tile([128, 1], FP, tag="m")
                    nc.scalar.mul(m[:], gsums[:, 0:1], 1.0 / N)
                    ex2 = smp.tile([128, 1], FP, tag="ex2")
                    nc.scalar.mul(ex2[:], gsums[:, 1:2], 1.0 / N)
                    var = smp.tile([128, 1], FP, tag="var")
                    nc.vector.tensor_mul(var[:], m[:], m[:])
                    nc.vector.tensor_sub(var[:], ex2[:], var[:])
                    sd = smp.tile([128, 1], FP, tag="sd")
                    nc.scalar.activation(sd[:], var[:],
                                         mybir.ActivationFunctionType.Sqrt,
                                         bias=eps_t[:])
                    inv = smp.tile([128, 1], FP, tag="inv")
                    nc.vector.reciprocal(inv[:], sd[:])
                    sc_l = smp.tile([128, 1], FP, tag="scl")
                    nc.vector.tensor_mul(sc_l[:], inv[:],
                                         bn_t[:, 2 * l:2 * l + 1])
                    bi_l = smp.tile([128, 1], FP, tag="bil")
                    nc.vector.tensor_mul(bi_l[:], m[:], sc_l[:])
                    nc.vector.tensor_sub(bi_l[:], bn_t[:, 2 * l + 1:2 * l + 2],
                                         bi_l[:])
                    # ---- sigmoid -> next h ----
                    h_next = hp.tile([128, SLP], BF, tag="h", bufs=2)
                    nc.scalar.activation(h_next[:], accum[:],
                                         mybir.ActivationFunctionType.Sigmoid,
                                         bias=bi_l[:], scale=sc_l[:])
                    h_cur = h_next
                # ---- pooling ----
                pps = psp.tile([G, 128], FP, tag="pool", bufs=1)
                for t in range(NT):
                    tp = psp.tile([128, 128], BF, tag="tp", bufs=2)
                    nc.tensor.transpose(
                        tp[:], h_cur[:, t * 128:(t + 1) * 128], id_t[:])
                    hn = msp.tile([128, 128], BF, tag="ms")
                    if t % 2 == 0:
                        nc.scalar.copy(hn[:], tp[:])
                    else:
                        nc.vector.tensor_copy(hn[:], tp[:])
                    nc.tensor.matmul(
                        pps[:], P_t[:, t * G:(t + 1) * G], hn[:],
                        start=(t == 0), stop=(t == NT - 1),
                    )
                po = smp.tile([G, 128], FP, tag="po")
                nc.vector.tensor_copy(po[:], pps[:])
                nc.sync.dma_start(out_d[:], po[:])
    nc.compile()
    return nc


# --------------------------------------------------------------------------
# Entry point
# --------------------------------------------------------------------------

def kernel(**inputs):
    cfg = DEFAULT_CFG
    x = np.asarray(inputs["x"], np.float32)
    edge_index = np.asarray(inputs["edge_index"])
    batch = np.asarray(inputs["batch"])
    args = [x, edge_index, batch] + [
        np.asarray(inputs[k], np.float32) for k in
        ("W1", "W2", "W3", "g1", "be1", "g2", "be2", "g3", "be3")]
    meta, in_maps = host_prep(cfg, *args)
    nc = build_gcn(cfg, meta, reps=1)
    res = run_bass_kernel_spmd(nc, in_maps, core_ids=list(range(cfg.n_cores)))
    pooled = np.zeros((cfg.G, 128), np.float64)
    for c in range(cfg.n_cores):
        pooled += res.results[c]["out"].astype(np.float64)
    cnt = np.bincount(np.asarray(batch, np.int64), minlength=cfg.G).astype(np.float64)
    out = pooled / np.maximum(cnt, 1.0)[:, None]
    return out.astype(np.float32)

